# revision 4
# baseline (speedup 1.0000x reference)
"""Category-equality Gram matrix for TRN2 via K=128 stacked one-hot matmul.

out[i, j] = 1.0 if Z[i] == Z[j] else 0.0, Z: [16384] int labels in [0, 64).

The reference is oh @ oh.T (a one-hot matmul with K=64 classes). The device
computes the same product with a fixed bit-packing matrix folded into the
right-hand side, 16 output bits per PSUM value:

    packed16[i, c] = sum_{b<16} 2^b * (Z[16c+b] == Z[i])
                   = onehot(Z[i]) . XpLo[:, c] + 256 * onehot(Z[i]) . XpHi[:, c]
    XpLo[k, c] = sum_{b<8} 2^b * (Z[16c+b]   == k)   (host-precomputed, [64, 1024])
    XpHi[k, c] = sum_{b<8} 2^b * (Z[16c+8+b] == k)

Since the contraction only needs 64 of the PE's 128 rows, the two one-hot
blocks are STACKED along K: stationary wst = [[onehotT(rows)], [256*onehotT]]
([128, 2048] bf16), moving xst = [[XpLo], [XpHi]] ([128, 1024] bf16). Every
value is an exact integer <= 65535 in bf16/f32 arithmetic (one-hot selects a
single entry), so the result is bit-exact. K=128 also keeps the PE array
fully occupied (warm 2.4 GHz HAM clock).

Pipeline per core (trace-derived; the wall is the 4 MiB HBM store stream at
~280 GB/s under 8-core contention, NOT compute):
  - inputs chunked across both HWDGE rings so the first matmul starts ~1.5 us
    into the kernel instead of ~5.5 (store stream starts ~4 us earlier).
  - per row tile: 2 matmuls (N=512) into ONE 2-bank PSUM tile [128, 1024] f32,
    then ONE 1024-wide PSUM->SBUF u16 cast (fixed overhead amortized:
    (172+1024)/1.2GHz ACT, (120+1024)/0.96GHz DVE), alternating ACT/DVE.
  - output DRAM layout is partition-major [128, 16*1024]: row-tile t lands in
    columns [t*1024,(t+1)*1024), so two adjacent row tiles form a 4 KiB/
    partition contiguous store. 8 store DMAs (vs 16) all on the sync ring,
    each 128 descriptors x 4 KiB. Host decode transposes back (cheap).

Per core: 4 MiB shipped = 1 bit per output element (the shipping floor).
The host reinterprets little-endian u16 as bytes, np.unpackbits, -> f32
(an exact, purely elementwise decode).

Row-parallel: core i computes rows [i*2048, (i+1)*2048).
"""

import numpy as np
import ml_dtypes

import concourse.tile as tile
from concourse import bacc, mybir
from concourse.bass_utils import run_bass_kernel_spmd

N = 16384          # labels / output dim
M = 8              # cores
RPC = N // M       # 2048 rows per core
P = 128            # SBUF partitions / PE output rows
T = RPC // P       # 16 row tiles per core
K = 64             # number of classes
KS = 128           # stacked contraction dim
NB = N // 16       # 1024 packed u16 per row
MMF = 512          # moving free dim per matmul (one PSUM bank of f32)
G = 2              # row tiles per store DMA

BF16 = mybir.dt.bfloat16
_NC_CACHE = None


def _build_nc():
    nc = bacc.Bacc("TRN2", target_bir_lowering=False, debug=False, num_devices=M)
    w = nc.dram_tensor("w", [KS, RPC], BF16, kind="ExternalInput").ap()
    x = nc.dram_tensor("x", [KS, NB], BF16, kind="ExternalInput").ap()
    # partition-major output: column block [t*NB, (t+1)*NB) holds row tile t
    out = nc.dram_tensor("out", [P, T * NB], mybir.dt.uint16,
                         kind="ExternalOutput").ap()

    with tile.TileContext(nc) as tc:
        with tc.tile_pool(name="inp", bufs=1) as inp, \
             tc.tile_pool(name="pp", bufs=3, space="PSUM") as pp, \
             tc.tile_pool(name="wu", bufs=1, space="PSUM") as wu, \
             tc.tile_pool(name="op", bufs=3) as op:
            w_s = inp.tile([KS, RPC], BF16, tag="w")
            x_s = inp.tile([KS, NB], BF16, tag="x")
            scr = inp.tile([P, MMF], BF16, tag="scr")
            # Chunked input loads, interleaved across both HWDGE rings so the
            # first matmul's deps land earliest:
            #   scalar: w tile0 (32K), w tiles 1-3 (96K)
            #   sync:   x lo half (128K), x hi half (128K), w tail (384K)
            nc.scalar.dma_start(w_s[:, 0:P], w[:, 0:P])
            nc.sync.dma_start(x_s[:, 0:MMF], x[:, 0:MMF])
            nc.scalar.dma_start(w_s[:, P:4 * P], w[:, P:4 * P])
            nc.sync.dma_start(x_s[:, MMF:NB], x[:, MMF:NB])
            nc.sync.dma_start(w_s[:, 4 * P:RPC], w[:, 4 * P:RPC])

            # HAM warm-up: matmuls on a memset scratch while inputs stream in,
            # so the PE clock gate is at 2.4 GHz when the real stream starts.
            nc.gpsimd.memset(scr[:], 0)
            wps = wu.tile([P, MMF], mybir.dt.float32, tag="wps")
            for _ in range(4):
                nc.tensor.matmul(wps[:], scr[:, 0:P], scr[:], start=True,
                                 stop=True)

            store_rings = [nc.sync, nc.scalar]
            ob = None
            for t in range(T):
                wt = w_s[:, t * P:(t + 1) * P]
                ps = pp.tile([P, NB], mybir.dt.float32, tag="ps")
                nc.tensor.matmul(ps[:, 0:MMF], wt, x_s[:, 0:MMF],
                                 start=True, stop=True)
                nc.tensor.matmul(ps[:, MMF:NB], wt, x_s[:, MMF:NB],
                                 start=True, stop=True)
                if t % G == 0:
                    ob = op.tile([P, G * NB], mybir.dt.uint16, tag="ob")
                obj = ob[:, (t % G) * NB:(t % G + 1) * NB]
                # one 1024-wide cast per row tile, alternating engines
                if t % 2 == 0:
                    nc.scalar.copy(obj, ps[:])
                else:
                    nc.vector.tensor_copy(obj, ps[:])
                if t % G == G - 1:
                    g = t // G
                    store_rings[g % 2].dma_start(
                        out[:, g * G * NB:(g + 1) * G * NB], ob[:])
    nc.compile()
    return nc


def _get_nc():
    global _NC_CACHE
    if _NC_CACHE is None:
        _NC_CACHE = _build_nc()
    return _NC_CACHE


def _in_maps(Z: np.ndarray) -> list[dict[str, np.ndarray]]:
    zflat = Z.reshape(-1).astype(np.int64)
    onehot = (np.arange(K)[:, None] == zflat[None, :])  # [K, N] bool
    pw = (1 << np.arange(8)).astype(np.int64)
    grp = onehot.reshape(K, NB, 16)
    xl = (grp[:, :, 0:8] * pw).sum(-1)
    xh = (grp[:, :, 8:16] * pw).sum(-1)
    bf = ml_dtypes.bfloat16
    xst = np.ascontiguousarray(np.concatenate([xl, xh], axis=0).astype(bf))
    maps = []
    for i in range(M):
        ohi = onehot[:, i * RPC:(i + 1) * RPC]
        wst = np.ascontiguousarray(
            np.concatenate([ohi, 256 * ohi.astype(np.int64)], axis=0).astype(bf)
        )
        maps.append({"w": wst, "x": xst})
    return maps


def kernel(Z: np.ndarray, **_ignored) -> np.ndarray:
    Z = np.asarray(Z).reshape(-1)
    assert Z.shape == (N,), Z.shape
    nc = _get_nc()
    res = run_bass_kernel_spmd(nc, _in_maps(Z), list(range(M)))
    out = np.empty((N, N), dtype=np.float32)
    for i in range(M):
        pm = res.results[i]["out"]  # [P, T*NB] u16, partition-major
        # [P, T, NB] -> [T, P, NB] -> [RPC, NB] row-major packed
        packed = np.ascontiguousarray(
            pm.reshape(P, T, NB).transpose(1, 0, 2).reshape(RPC, NB))
        by = packed.view(np.uint8)  # [RPC, 2*NB] little-endian
        bits = np.unpackbits(by, axis=1, bitorder="little")  # [RPC, N]
        out[i * RPC:(i + 1) * RPC] = bits
    return out
